# revision 10
# baseline (speedup 1.0000x reference)
"""Bahdanau additive attention, data-parallel over batch on 8 TRN2 NeuronCores.

Math (per batch row b):
    dec_proj = W @ prev[b] + b_W                       # [A]   (computed on host: tiny)
    enc_proj[s] = U @ enc[b,s] + b_U                   # [S, A]
    energy[s] = v . tanh(dec_proj + enc_proj[s])       # [S]
    w = exp(energy);  c[b] = (w @ enc[b]) / sum(w)     # [CTX]

Device strategy (per core, 8 batches):
  - enc passed as bf16.  For each 128-row s-tile:
      * natural DMA      -> [s=128, c=1024]  (rhs for the weighted-sum matmul)
      * XBAR transpose   -> [c=128 x 8, s=128] (stationary lhsT for the U-projection)
  - U-projection: psum[s,A] = sum_k trT[c_k,s].T @ UT[c_k,A], bias folded in via a
    K=1 ones-matmul with rhs = (dec_proj[b] + b_U).
  - tanh on ScalarE, v-weighting on VectorE, sum_a via ScalarE Copy+accum.
  - exp -> w (bf16); weighted sum + denominator accumulate in PSUM across the
    32 s-tiles of a batch:  c_psum[1,512]x2 += w.T @ enc_tile, den += w.T @ ones.
  - epilogue per batch: c = c_psum * (1/den), DMA out fp32.
One HBM pass of enc per layout (2 reads of the bf16 tensor total).
"""

import sys

sys.path.insert(0, "/opt/trn_rl_repo")

import numpy as np
import ml_dtypes

import concourse.bass as bass
from concourse import bacc
import concourse.mybir as mybir
import concourse.tile as tile
from concourse.bass_utils import run_bass_kernel_spmd

B, S, A, DD, CTX = 64, 4096, 256, 1024, 1024
NCORES = 8
BL = B // NCORES  # 8 batches per core
P = 128
KC = CTX // P  # 8 contraction chunks
MT = S // P    # 32 s-tiles per batch
BF16 = mybir.dt.bfloat16
F32 = mybir.dt.float32

_CACHE = {}


def _fast_bf16(x: np.ndarray) -> np.ndarray:
    """float32 -> bfloat16 with round-to-nearest-even via integer ops
    (ml_dtypes.astype is ~50x slower on GiB-scale arrays)."""
    u = np.ascontiguousarray(x, dtype=np.float32).view(np.uint32)
    r = ((u + 0x7FFF + ((u >> 16) & 1)) >> 16).astype(np.uint16)
    return r.view(ml_dtypes.bfloat16)


def _build():
    nc = bacc.Bacc()
    enc = nc.declare_dram_parameter("enc", [BL, S, CTX], BF16, isOutput=False)
    enct = nc.declare_dram_parameter("enct", [BL, CTX, S], BF16, isOutput=False)
    ut = nc.declare_dram_parameter("ut", [CTX, A], BF16, isOutput=False)
    db = nc.declare_dram_parameter("db", [BL, A], BF16, isOutput=False)
    v = nc.declare_dram_parameter("vv", [A], BF16, isOutput=False)
    out = nc.declare_dram_parameter("out", [BL, CTX], F32, isOutput=True)

    ST = 512          # s-rows per super-tile (one ~1MB DMA per layout)
    NSUB = ST // P    # 4 proj subtiles per super-tile
    NSUP = S // ST    # 8 super-tiles per batch

    with tile.TileContext(nc) as tc:
        with (
            tc.tile_pool(name="const", bufs=1) as const,
            tc.tile_pool(name="work", bufs=4) as work,
            tc.tile_pool(name="psum", bufs=3, space="PSUM") as psum,
            tc.tile_pool(name="acc", bufs=1, space="PSUM") as accp,
        ):
            # ---- constants, loaded once ----
            ut_sb = const.tile([P, KC, A], BF16)
            nc.sync.dma_start(ut_sb[:], ut.rearrange("(k p) a -> p k a", p=P))
            db_sb = const.tile([1, BL * A], BF16)
            nc.sync.dma_start(db_sb[:], db.rearrange("b a -> (b a)")[None, :])
            v_sb = const.tile([1, A], BF16)
            nc.sync.dma_start(v_sb[:], v[None, :])
            ones_row = const.tile([1, P], BF16)
            nc.vector.memset(ones_row[:], 1.0)
            ones_col = const.tile([P, 1], BF16)
            nc.vector.memset(ones_col[:], 1.0)
            zbias0 = const.tile([P, 1], F32)
            nc.vector.memset(zbias0[:], 0.0)

            # v replicated to all 128 partitions: ones_row.T @ v_sb
            vrep_ps = accp.tile([P, A], F32, tag="vrep")
            nc.tensor.matmul(vrep_ps[:], ones_row[:], v_sb[:], start=True, stop=True)
            # Walrus allows only ONE sync-wait per Activation instruction, so the
            # ScalarE stream is sequenced to observe the PE and DVE clocks up
            # front: (1) the v_rep copy waits on PE, (2) the zbias_act copy
            # waits on DVE.  The steady-state Tanh then needs only its PSUM
            # (PE) wait, and Exp's bias (zbias_act) keeps its deps on the ACT
            # semaphore where they merge into a single wait.
            v_rep = const.tile([P, A], F32)
            nc.scalar.activation(v_rep[:], vrep_ps[:],
                                 mybir.ActivationFunctionType.Copy)
            zbias = const.tile([P, 1], F32)
            nc.scalar.activation(zbias[:], zbias0[:],
                                 mybir.ActivationFunctionType.Copy)

            enct3 = enct.rearrange("b (k p) s -> b p k s", p=P)
            for b in range(BL):
                c0 = accp.tile([1, 512], F32, tag="c0")
                c1 = accp.tile([1, 512], F32, tag="c1")
                den = accp.tile([1, 1], F32, tag="den")
                for t in range(NSUP):
                    s0 = t * ST
                    nat = work.tile([P, NSUB, CTX], BF16, tag="nat")
                    nc.sync.dma_start(
                        nat[:],
                        enc[b, s0:s0 + ST, :].rearrange("(o p) c -> p o c", p=P))
                    tr = work.tile([P, KC, ST], BF16, tag="tr")
                    nc.sync.dma_start(tr[:], enct3[b, :, :, s0:s0 + ST])

                    for u in range(NSUB):
                        j = t * NSUB + u
                        # projection + bias into PSUM [s=128, A]
                        proj = psum.tile([P, A], F32, tag="proj")
                        nc.tensor.matmul(
                            proj[:], ones_row[:], db_sb[:, b * A:(b + 1) * A],
                            start=True, stop=False,
                        )
                        for k in range(KC):
                            nc.tensor.matmul(
                                proj[:], tr[:, k, u * P:(u + 1) * P],
                                ut_sb[:, k, :],
                                start=False, stop=(k == KC - 1),
                            )

                        th = work.tile([P, A], F32, tag="th")
                        nc.scalar.activation(
                            th[:], proj[:], mybir.ActivationFunctionType.Tanh,
                            bias=zbias0[:],
                        )
                        ew = work.tile([P, A], F32, tag="ew")
                        nc.vector.tensor_mul(out=ew[:], in0=th[:], in1=v_rep[:])
                        dump = work.tile([P, A], BF16, tag="dump")
                        energy = work.tile([P, 1], F32, tag="energy")
                        nc.scalar.activation(
                            dump[:], ew[:], mybir.ActivationFunctionType.Copy,
                            accum_out=energy[:],
                        )
                        wexp = work.tile([P, 1], BF16, tag="wexp")
                        nc.scalar.activation(
                            wexp[:], energy[:], mybir.ActivationFunctionType.Exp,
                            bias=zbias[:],
                        )

                        first, last = (j == 0), (j == MT - 1)
                        nc.tensor.matmul(c0[:], wexp[:], nat[:, u, 0:512],
                                         start=first, stop=last)
                        nc.tensor.matmul(c1[:], wexp[:], nat[:, u, 512:1024],
                                         start=first, stop=last)
                        nc.tensor.matmul(den[:], wexp[:], ones_col[:],
                                         start=first, stop=last)

                rec = work.tile([1, 1], F32, tag="rec")
                nc.vector.reciprocal(rec[:], den[:])
                cout = work.tile([1, CTX], F32, tag="cout")
                nc.vector.tensor_scalar_mul(cout[:, 0:512], c0[:], rec[:])
                nc.vector.tensor_scalar_mul(cout[:, 512:1024], c1[:], rec[:])
                nc.sync.dma_start(out[b][None, :], cout[:])

    if not nc.is_finalized():
        nc.finalize()
    return nc


def kernel(previous_decoder_hidden_state, encoder_final_hidden_layers,
           W, b_W, U, b_U, v):
    prev = np.asarray(previous_decoder_hidden_state, dtype=np.float32)
    enc = np.asarray(encoder_final_hidden_layers, dtype=np.float32)
    W = np.asarray(W, dtype=np.float32)
    b_W = np.asarray(b_W, dtype=np.float32)
    U = np.asarray(U, dtype=np.float32)
    b_U = np.asarray(b_U, dtype=np.float32)
    v = np.asarray(v, dtype=np.float32)

    if "nc" not in _CACHE:
        _CACHE["nc"] = _build()
    nc = _CACHE["nc"]

    # host-side prep (tiny, except the enc cast which uses a fast bit path)
    db = (prev @ W.T + b_W + b_U).astype(ml_dtypes.bfloat16)   # [B, A]
    ut = np.ascontiguousarray(U.T).astype(ml_dtypes.bfloat16)  # [CTX, A]
    enc_bf = _fast_bf16(enc)                                   # [B, S, CTX]
    enct_bf = np.ascontiguousarray(enc_bf.transpose(0, 2, 1))  # [B, CTX, S]
    v_bf = v.astype(ml_dtypes.bfloat16)

    in_maps = []
    for i in range(NCORES):
        sl = slice(i * BL, (i + 1) * BL)
        in_maps.append({
            "enc": enc_bf[sl],
            "enct": enct_bf[sl],
            "ut": ut,
            "db": db[sl],
            "vv": v_bf,
        })

    res = run_bass_kernel_spmd(nc, in_maps, list(range(NCORES)),
                               **_CACHE.get("run_kwargs", {}))
    _CACHE["last_result"] = res
    outs = [np.asarray(r["out"]) for r in res.results]
    return np.concatenate(outs, axis=0).astype(np.float32)
